# revision 3
# baseline (speedup 1.0000x reference)
"""Trainium2 Bass kernel for nn_CombinedModel (3-relation GNN with Bernstein
polynomial message passing).

v2: identity-round scatter. dinv[src] is folded into the AllGathered table
(scaled during the PSUM->nm transpose copy) and dinv[dst] into a per-window
diagonal rhs, so most edges are aggregated by matmuls against a constant
diagonal (no per-tile one-hot build). Host assigns each edge of dst slot q to
round r (its r-th edge), so round tiles are slot-aligned: gathered row e
belongs to dst slot e. Empty slots gather a dedicated zero row (ghost slot,
zeroed via dinv=0). Leftover edges (beyond the per-window round count R) use
the one-hot path (is_equal vs iota, scaled by dinv[dst]), pinned to DVE.
W3 projections are fused into the per-seg hop epilogue; T2 is never
materialized in full.
"""
import math
import os
from contextlib import ExitStack

import numpy as np

import concourse.bacc as bacc
import concourse.tile as tile
from concourse import mybir
from concourse.bass_utils import run_bass_kernel_spmd
from concourse.masks import make_identity

F16, F32 = mybir.dt.float16, mybir.dt.float32
I16, I32 = mybir.dt.int16, mybir.dt.int32

NCORES = 8
P = 128
H = 128
IN_FEATS = 256
R = 3
D_ORDER = 2
KORD = D_ORDER + 1
WIN_PER_SEG = 4
MLP_CHUNK = 512
SPLIT = 25600        # flat-row lo/hi split (int16 index range)
Q0 = 0.5             # min fill fraction for an identity round


def _bernstein_thetas(d):
    thetas = []
    for i in range(d + 1):
        a = np.zeros(i + 1)
        a[i] = 0.5 ** i
        b = np.array([math.comb(d - i, j) * (-0.5) ** j for j in range(d - i + 1)])
        scale = math.factorial(d + 1) / (math.factorial(i) * math.factorial(d - i))
        thetas.append((np.convolve(a, b) * scale).astype(np.float32))
    return np.stack(thetas)  # [d+1, d+1]


THETAS = _bernstein_thetas(D_ORDER)


# ----------------------------------------------------------------------------
# Host-side preprocessing
# ----------------------------------------------------------------------------

def _make_plan(n):
    nloc = ((n + NCORES * P - 1) // (NCORES * P)) * P
    npad = nloc * NCORES
    nwin = nloc // P
    segs = [list(range(s, min(s + WIN_PER_SEG, nwin)))
            for s in range(0, nwin, WIN_PER_SEG)]
    return dict(N=n, NLOC=nloc, NPAD=npad, NWIN=nwin, segs=segs)


def _build_perm(degs_total, npad):
    """Balanced node -> slot permutation (snake-deal by total degree), with
    the last slot of the last window of each core reserved for a ghost
    (zero) node used as the gather target for empty round slots."""
    n = len(degs_total)
    nloc = npad // NCORES
    nwin = nloc // P
    tot = np.zeros(npad, np.int64)
    tot[:n] = degs_total
    order = np.argsort(-tot, kind="stable")
    slot_of = np.empty(npad, np.int64)
    counts = np.zeros(npad // P, np.int64)
    i = np.arange(npad)
    rnd, pos = np.divmod(i, npad // P)
    w = np.where(rnd % 2 == 0, pos, npad // P - 1 - pos)
    core = w % NCORES
    j = w // NCORES
    for idx in range(npad):
        g = order[idx]
        ww = w[idx]
        slot_of[g] = core[idx] * nloc + j[idx] * P + counts[ww]
        counts[ww] += 1
    # reserve ghost slots: swap ghosts (ids npad-1..npad-NCORES) into the
    # fixed slot (core c, window nwin-1, q=P-1)
    assert npad - n >= NCORES
    inv = np.empty(npad, np.int64)
    inv[slot_of] = np.arange(npad)
    for c in range(NCORES):
        tgt = c * nloc + (nwin - 1) * P + (P - 1)
        gid = npad - 1 - c
        cur = slot_of[gid]
        if cur == tgt:
            continue
        other = inv[tgt]
        slot_of[other], slot_of[gid] = cur, tgt
        inv[cur], inv[tgt] = other, gid
    return slot_of


def _pack_idx(idx_flat):
    """[L] int16 -> wrapped [128, L//16] layout (16-partition wrap, replicated)."""
    L = len(idx_flat)
    assert L % 16 == 0
    base = idx_flat.reshape(L // 16, 16).T  # [16, L/16]
    return np.ascontiguousarray(np.tile(base, (8, 1))).astype(np.int16)


def _flat_row(slot, nloc, nwin):
    """node slot (global) -> flat 256B-row index in the wrapped AG table.

    slot = c*nloc + j*128 + q  ->  row (c*128 + q)*nwin + j
    """
    c = slot // nloc
    l = slot % nloc
    j = l // P
    q = l % P
    return (c * P + q) * nwin + j


def _ghost_rows(nwin):
    lo = (0 * P + (P - 1)) * nwin + (nwin - 1)
    hi = ((NCORES - 1) * P + (P - 1)) * nwin + (nwin - 1) - SPLIT
    assert 0 <= lo < SPLIT and 0 <= hi < 32768
    return lo, hi


def _build_streams(plan, s_slot, d_slot, dinv_dst_edge):
    """Identity-round + one-hot-tail streams for one relation.

    Returns (Rr [nwin,2], Tt [nwin,2], ntiles, ntail, per_core list of
    dict(idx, dq, wq)). Stream tile order: for seg: for part: for w in seg:
    [R ident tiles][T tail tiles]. Ident tile r holds, at partition q, the
    src row of dst-slot q's r-th edge on that part (or the ghost zero row).
    Tail edges are compacted (dq = dst slot in window, wq = dinv[dst]).
    """
    NLOC, NWIN, segs = plan["NLOC"], plan["NWIN"], plan["segs"]
    ghost_lo, ghost_hi = _ghost_rows(NWIN)
    ghost = (ghost_lo, ghost_hi)

    srow = _flat_row(s_slot, NLOC, NWIN)
    part = (srow >= SPLIT).astype(np.int64)
    ival = (srow - part * SPLIT).astype(np.int64)
    core = d_slot // NLOC
    lloc = d_slot % NLOC
    w = lloc // P
    q = lloc % P

    key = ((core * NWIN + w) * 2 + part) * P + q
    nkey = NCORES * NWIN * 2 * P
    cnt = np.bincount(key, minlength=nkey).reshape(NCORES, NWIN, 2, P)

    Rr = np.zeros((NWIN, 2), np.int64)
    for wi in range(NWIN):
        for pt in (0, 1):
            cs = cnt[:, wi, pt, :]
            r = 0
            while (cs >= r + 1).sum() >= Q0 * NCORES * P:
                r += 1
            Rr[wi, pt] = r
    leftover = np.maximum(cnt - Rr[None, :, :, None], 0).sum(axis=3)  # [C,NWIN,2]
    Tt = np.ceil(leftover / P).astype(np.int64).max(axis=0)           # [NWIN,2]

    tile_off = {}
    tail_off = {}
    tpos = tc = 0
    for seg in segs:
        for pt in (0, 1):
            for wi in seg:
                tile_off[(wi, pt)] = tpos
                tail_off[(wi, pt)] = tc
                tpos += int(Rr[wi, pt] + Tt[wi, pt])
                tc += int(Tt[wi, pt])
    ntiles, ntail = tpos, tc

    # rank of each edge within its (c,w,part,q) group, ordered by src row
    order = np.lexsort((ival, q, part, w, core))
    k_sorted = key[order]
    new_grp = np.r_[True, np.diff(k_sorted) != 0]
    starts = np.flatnonzero(new_grp)
    grp_id = np.cumsum(new_grp) - 1
    rank = np.arange(len(order)) - starts[grp_id]

    per_core = []
    for c in range(NCORES):
        sel = order[core[order] == c]
        rk = rank[core[order] == c]
        idx_c = np.empty(ntiles * P, np.int64)
        # default fill: ghost row of the matching part per tile
        for (wi, pt), t0 in tile_off.items():
            nt = int(Rr[wi, pt] + Tt[wi, pt])
            idx_c[t0 * P:(t0 + nt) * P] = ghost[pt]
        dq_c = np.full(max(ntail, 1) * P, -1.0, np.float32)
        wq_c = np.zeros(max(ntail, 1) * P, np.float32)

        is_ident = rk < Rr[w[sel], part[sel]]
        e_id = sel[is_ident]
        pos = (np.array([tile_off[(wi, pt)] for wi, pt in
                         zip(w[e_id], part[e_id])], np.int64)
               + rk[is_ident]) * P + q[e_id] if len(e_id) else np.empty(0, np.int64)
        idx_c[pos] = ival[e_id]

        e_tail = sel[~is_ident]
        if len(e_tail):
            # group tails by (w, part); order within group by src row
            tkey = w[e_tail] * 2 + part[e_tail]
            t_ord = np.lexsort((ival[e_tail], tkey))
            e_tail = e_tail[t_ord]
            tkey = tkey[t_ord]
            tnew = np.r_[True, np.diff(tkey) != 0]
            tstarts = np.flatnonzero(tnew)
            tgrp = np.cumsum(tnew) - 1
            tr = np.arange(len(e_tail)) - tstarts[tgrp]
            base_tile = np.array(
                [tile_off[(wi, pt)] + int(Rr[wi, pt]) for wi, pt in
                 zip(w[e_tail], part[e_tail])], np.int64)
            tpos_e = (base_tile + tr // P) * P + tr % P
            idx_c[tpos_e] = ival[e_tail]
            base_col = np.array(
                [tail_off[(wi, pt)] for wi, pt in
                 zip(w[e_tail], part[e_tail])], np.int64)
            cpos = (base_col + tr // P) * P + tr % P
            dq_c[cpos] = q[e_tail].astype(np.float32)
            wq_c[cpos] = dinv_dst_edge[e_tail]

        per_core.append(dict(
            idx=_pack_idx(idx_c.astype(np.int16)),
            dq=np.ascontiguousarray(dq_c.reshape(max(ntail, 1), P).T),
            wq=np.ascontiguousarray(wq_c.reshape(max(ntail, 1), P).T),
        ))
    return Rr, Tt, ntiles, ntail, per_core


def preprocess(inputs):
    x = np.asarray(inputs["x"], np.float32)
    n = x.shape[0]
    plan = _make_plan(n)
    NLOC, NPAD, NWIN = plan["NLOC"], plan["NPAD"], plan["NWIN"]

    srcs, dsts, degs = [], [], []
    for r in range(R):
        s = np.asarray(inputs[f"src{r}"]).astype(np.int64)
        d = np.asarray(inputs[f"dst{r}"]).astype(np.int64)
        srcs.append(s)
        dsts.append(d)
        degs.append(np.bincount(d, minlength=n).astype(np.float64))
    perm = _build_perm(sum(degs)[:n].astype(np.int64), NPAD)  # global -> slot
    inv_perm = np.empty(NPAD, np.int64)
    inv_perm[perm] = np.arange(NPAD)

    meta = dict(N=n, NLOC=NLOC, NPAD=NPAD, NWIN=NWIN,
                segs=tuple(tuple(s) for s in plan["segs"]))
    Rs_l, Ts_l, ntiles_l, ntail_l, streams, dinv_nms = [], [], [], [], [], []
    for r in range(R):
        dinv = (1.0 / np.sqrt(np.maximum(degs[r], 1.0))).astype(np.float32)
        Rr, Tt, ntiles, ntail, per_core = _build_streams(
            plan, perm[srcs[r]], perm[dsts[r]], dinv[dsts[r]])
        Rs_l.append(tuple(tuple(int(v) for v in row) for row in Rr))
        Ts_l.append(tuple(tuple(int(v) for v in row) for row in Tt))
        ntiles_l.append(ntiles)
        ntail_l.append(max(ntail, 1))
        streams.append(per_core)
        # per-core dinv in wrapped (q, window) layout; 0 for pad slots
        dinv_pad = np.zeros(NPAD, np.float32)
        dinv_pad[:n] = dinv[:n]
        dn = []
        for c in range(NCORES):
            slots = (c * NLOC + np.arange(NLOC)).reshape(NWIN, P)
            vals = dinv_pad[np.minimum(inv_perm[slots], n - 1)]
            vals[inv_perm[slots] >= n] = 0.0
            dn.append(np.ascontiguousarray(vals.T.astype(np.float32)))  # [P,NWIN]
        dinv_nms.append(dn)
    meta["R"] = tuple(Rs_l)
    meta["T"] = tuple(Ts_l)
    meta["ntiles"] = tuple(ntiles_l)
    meta["ntail"] = tuple(ntail_l)

    x_slots = np.zeros((NPAD, IN_FEATS), np.float32)
    x_slots[perm[:n]] = x
    in_maps = []
    weight_names = []
    for r in range(R):
        weight_names += [f"W1_{r}", f"b1_{r}", f"W2_{r}", f"b2_{r}"]
    weight_names += ["W3", "b3"]
    for c in range(NCORES):
        m = {"xT": np.ascontiguousarray(
            x_slots[c * NLOC:(c + 1) * NLOC].T)}
        for name in weight_names:
            m[name] = np.asarray(inputs[name], np.float32)
        for r in range(R):
            m[f"idx{r}"] = streams[r][c]["idx"]
            m[f"dq{r}"] = streams[r][c]["dq"]
            m[f"wq{r}"] = streams[r][c]["wq"]
            m[f"dinv{r}"] = dinv_nms[r][c]
        in_maps.append(m)
    return meta, in_maps, perm


# ----------------------------------------------------------------------------
# Device program
# ----------------------------------------------------------------------------

def build_program(meta):
    NLOC, NPAD, NWIN = meta["NLOC"], meta["NPAD"], meta["NWIN"]
    segs = [list(s) for s in meta["segs"]]
    Rs = [np.array(t, np.int64) for t in meta["R"]]
    Ts = [np.array(t, np.int64) for t in meta["T"]]
    ntiles = meta["ntiles"]
    ntail = meta["ntail"]

    # per-(seg,part) gather tile counts and max for vb sizing
    def seg_part_tiles(r, seg, pt):
        return int(sum(Rs[r][wi, pt] + Ts[r][wi, pt] for wi in seg))

    vbw = 1
    for r in range(R):
        for seg in segs:
            for pt in (0, 1):
                vbw = max(vbw, seg_part_tiles(r, seg, pt))

    # tile/tail offset bookkeeping (must match host stream order)
    tile_offs, tail_offs = [], []
    for r in range(R):
        to, ta = {}, {}
        tpos = tc = 0
        for seg in segs:
            for pt in (0, 1):
                for wi in seg:
                    to[(wi, pt)] = tpos
                    ta[(wi, pt)] = tc
                    tpos += int(Rs[r][wi, pt] + Ts[r][wi, pt])
                    tc += int(Ts[r][wi, pt])
        assert tpos == ntiles[r]
        tile_offs.append(to)
        tail_offs.append(ta)

    nc = bacc.Bacc("TRN2", target_bir_lowering=False, debug=False,
                   num_devices=NCORES)

    xT_d = nc.dram_tensor("xT", [IN_FEATS, NLOC], F32, kind="ExternalInput").ap()
    Wd = {}
    for r in range(R):
        Wd[f"W1_{r}"] = nc.dram_tensor(f"W1_{r}", [IN_FEATS, H], F32, kind="ExternalInput").ap()
        Wd[f"b1_{r}"] = nc.dram_tensor(f"b1_{r}", [H], F32, kind="ExternalInput").ap()
        Wd[f"W2_{r}"] = nc.dram_tensor(f"W2_{r}", [H, H], F32, kind="ExternalInput").ap()
        Wd[f"b2_{r}"] = nc.dram_tensor(f"b2_{r}", [H], F32, kind="ExternalInput").ap()
    W3_d = nc.dram_tensor("W3", [KORD * H, H], F32, kind="ExternalInput").ap()
    b3_d = nc.dram_tensor("b3", [H], F32, kind="ExternalInput").ap()
    idx_d, dq_d, wq_d, dinv_d = [], [], [], []
    for r in range(R):
        idx_d.append(nc.dram_tensor(f"idx{r}", [P, ntiles[r] * 8], I16, kind="ExternalInput").ap())
        dq_d.append(nc.dram_tensor(f"dq{r}", [P, ntail[r]], F32, kind="ExternalInput").ap())
        wq_d.append(nc.dram_tensor(f"wq{r}", [P, ntail[r]], F32, kind="ExternalInput").ap())
        dinv_d.append(nc.dram_tensor(f"dinv{r}", [P, NWIN], F32, kind="ExternalInput").ap())
    out_d = nc.dram_tensor("out", [P, NLOC], F32, kind="ExternalOutput").ap()

    aghin, htab, agtin, ttab = [], [], [], []
    for r in range(R):
        aghin.append(nc.dram_tensor(f"aghin{r}", [P, NLOC], F16))
        htab.append(nc.dram_tensor(f"htab{r}", [NCORES * P, NLOC], F16,
                                   addr_space="Shared"))
        agtin.append(nc.dram_tensor(f"agtin{r}", [P, NLOC], F16))
        ttab.append(nc.dram_tensor(f"ttab{r}", [NCORES * P, NLOC], F16,
                                   addr_space="Shared"))

    mlp_chunks = []
    c0 = 0
    while c0 < NLOC:
        cw = min(MLP_CHUNK, NLOC - c0)
        mlp_chunks.append((c0, cw))
        c0 += cw

    with tile.TileContext(nc) as tc_, ExitStack() as ctx:
        consts = ctx.enter_context(tc_.tile_pool(name="consts", bufs=1))
        wtmp_p = ctx.enter_context(tc_.tile_pool(name="wtmp", bufs=2))
        nm_p = ctx.enter_context(tc_.tile_pool(name="nm", bufs=2))
        idx_p = ctx.enter_context(tc_.tile_pool(name="idxp", bufs=1))
        dq_p = ctx.enter_context(tc_.tile_pool(name="dqp", bufs=1))
        v_p = ctx.enter_context(tc_.tile_pool(name="vp", bufs=4))
        m_p = ctx.enter_context(tc_.tile_pool(name="mp", bufs=6))
        x_p = ctx.enter_context(tc_.tile_pool(name="xp", bufs=2))
        h1_p = ctx.enter_context(tc_.tile_pool(name="h1p", bufs=2))
        t2_p = ctx.enter_context(tc_.tile_pool(name="t2p", bufs=2))
        oc_p = ctx.enter_context(tc_.tile_pool(name="ocp", bufs=1))
        cast_p = ctx.enter_context(tc_.tile_pool(name="castp", bufs=1))
        pp_big = ctx.enter_context(tc_.tile_pool(name="ppbig", bufs=3, space="PSUM"))
        pp_hop = ctx.enter_context(tc_.tile_pool(name="pphop", bufs=3, space="PSUM"))
        pp_tr = ctx.enter_context(tc_.tile_pool(name="pptr", bufs=2, space="PSUM"))

        # ---- constants ----
        iota_i = wtmp_p.tile([P, P], I32, tag="iotai")
        nc.gpsimd.iota(iota_i[:], pattern=[[1, P]], base=0,
                       channel_multiplier=0)
        iota_f = consts.tile([P, P], F16, tag="iotaf")
        nc.vector.tensor_copy(iota_f[:], iota_i[:])
        ident = consts.tile([P, P], F16, tag="ident")
        make_identity(nc, ident[:])

        def load_cast(dst, src_ap, ncols):
            c0 = 0
            while c0 < ncols:
                cw = min(512, ncols - c0)
                tmp = cast_p.tile([P, 512], F32, tag="cast")
                nc.sync.dma_start(out=tmp[:, 0:cw], in_=src_ap[:, c0:c0 + cw])
                nc.any.tensor_copy(dst[:, c0:c0 + cw], tmp[:, 0:cw])
                c0 += cw

        W1a, W1b, W2sb, b1c, b2c = [], [], [], [], []
        for r in range(R):
            wa = consts.tile([P, H], F16, tag=f"w1a{r}")
            wb = consts.tile([P, H], F16, tag=f"w1b{r}")
            w2 = consts.tile([P, H], F16, tag=f"w2{r}")
            load_cast(wa, Wd[f"W1_{r}"][0:P, :], H)
            load_cast(wb, Wd[f"W1_{r}"][P:2 * P, :], H)
            load_cast(w2, Wd[f"W2_{r}"][:, :], H)
            b1 = consts.tile([P, 1], F32, tag=f"b1{r}")
            b2 = consts.tile([P, 1], F32, tag=f"b2{r}")
            nc.sync.dma_start(out=b1[:], in_=Wd[f"b1_{r}"][:, None])
            nc.sync.dma_start(out=b2[:], in_=Wd[f"b2_{r}"][:, None])
            W1a.append(wa); W1b.append(wb); W2sb.append(w2)
            b1c.append(b1); b2c.append(b2)

        # W3 folded by Bernstein thetas: W3p_k = sum_j THETA[j,k] * W3_j
        w3s = []
        for jj in range(KORD):
            t = wtmp_p.tile([P, H], F32, tag=f"w3s{jj}")
            nc.sync.dma_start(out=t[:], in_=W3_d[jj * H:(jj + 1) * H, :])
            w3s.append(t)
        W3p = []
        for k in range(KORD):
            acc = wtmp_p.tile([P, H], F32, tag=f"w3acc{k}")
            nc.vector.tensor_scalar(out=acc[:], in0=w3s[0][:],
                                    scalar1=float(THETAS[0, k]), scalar2=None,
                                    op0=mybir.AluOpType.mult)
            for jj in range(1, KORD):
                t2t = wtmp_p.tile([P, H], F32, tag="w3mul")
                nc.vector.tensor_scalar(out=t2t[:], in0=w3s[jj][:],
                                        scalar1=float(THETAS[jj, k]), scalar2=None,
                                        op0=mybir.AluOpType.mult)
                nc.vector.tensor_tensor(out=acc[:], in0=acc[:], in1=t2t[:],
                                        op=mybir.AluOpType.add)
            wk = consts.tile([P, H], F16, tag=f"w3p{k}")
            nc.vector.tensor_copy(wk[:], acc[:])
            W3p.append(wk)
        b3x3 = consts.tile([P, 1], F32, tag="b3x3")
        nc.sync.dma_start(out=b3x3[:], in_=b3_d[:, None])
        nc.vector.tensor_scalar(out=b3x3[:], in0=b3x3[:], scalar1=3.0,
                                scalar2=None, op0=mybir.AluOpType.mult)

        out_acc = consts.tile([P, NLOC], F16, tag="outacc")

        hT, T1, dinv_sb = [], [], []
        for r in range(R):
            hT.append(consts.tile([P, NLOC], F16, tag=f"ht{r}"))
            T1.append(consts.tile([P, NLOC], F16, tag=f"t1{r}"))
            dv = consts.tile([P, NWIN], F32, tag=f"dinv{r}")
            nc.sync.dma_start(out=dv[:], in_=dinv_d[r][:])
            dinv_sb.append(dv)

        idx_sb, dq_sb, wq_sb = [], [], []
        for r in range(R):
            idx_r = idx_p.tile([P, ntiles[r] * 8], I16, tag=f"idx{r}")
            dq_r = dq_p.tile([P, ntail[r]], F32, tag=f"dq{r}")
            wq_r = dq_p.tile([P, ntail[r]], F32, tag=f"wq{r}")
            idx_sb.append(idx_r)
            dq_sb.append(dq_r)
            wq_sb.append(wq_r)
        for r in range(R):
            nc.sync.dma_start(out=idx_sb[r][:], in_=idx_d[r][:])
            nc.sync.dma_start(out=dq_sb[r][:], in_=dq_d[r][:])
            nc.sync.dma_start(out=wq_sb[r][:], in_=wq_d[r][:])

        def transpose_scale_window(src_fm, r, wi, nm_tile):
            """nm[:, wi*P:(wi+1)*P] = dinv[:,wi] * transpose(src window)."""
            tp = pp_tr.tile([P, P], F16, space="PSUM", tag="tr")
            nc.tensor.transpose(out=tp[:], in_=src_fm[:, wi * P:(wi + 1) * P],
                                identity=ident[:])
            nc.any.tensor_scalar(out=nm_tile[:, wi * P:(wi + 1) * P],
                                 in0=tp[:], scalar1=dinv_sb[r][:, wi:wi + 1],
                                 scalar2=None, op0=mybir.AluOpType.mult)

        def store_and_allgather(nm_tile, ag_in, table):
            nc.sync.dma_start(out=ag_in.ap(), in_=nm_tile[:])
            if os.environ.get("KNOCC"):
                return
            nc.gpsimd.collective_compute(
                "AllGather", mybir.AluOpType.bypass,
                ins=[ag_in.ap()], outs=[table.ap()],
                replica_groups=[list(range(NCORES))])

        def w3_accum_seg(src_ap, k, cols0, cw, first=False):
            psf = pp_big.tile([P, MLP_CHUNK], F32, space="PSUM", tag="big")
            nc.tensor.matmul(out=psf[:, 0:cw], lhsT=W3p[k][:], rhs=src_ap,
                             start=True, stop=True)
            if first:
                nc.any.tensor_copy(out_acc[:, cols0:cols0 + cw], psf[:, 0:cw])
            else:
                nc.scalar.tensor_tensor(out=out_acc[:, cols0:cols0 + cw],
                                        in0=out_acc[:, cols0:cols0 + cw],
                                        in1=psf[:, 0:cw],
                                        op=mybir.AluOpType.add)

        def hop(r, table, prev_fm, k_w3, t1_out):
            """One hop: gather + diag/one-hot matmuls + fused epilogue.

            If t1_out is not None: writes T1 and also builds+stores the
            scaled nm for the next AllGather (returns nm tile). Otherwise
            uses a transient seg tile (hop2) and only accumulates W3.
            """
            Rr, Tt = Rs[r], Ts[r]
            to, ta = tile_offs[r], tail_offs[r]
            kmode = os.environ.get("KMODE", "full")
            nm_tile = None
            if t1_out is not None:
                nm_tile = nm_p.tile([P, NLOC], F16, tag="nm")
            flat = table.ap().rearrange("a (j f) -> (a j) f", f=H)
            bases = (flat[0:SPLIT, :], flat[SPLIT:NPAD, :])
            icol = 0
            for seg in segs:
                segw = len(seg) * P
                j0 = seg[0]
                vbs = {}
                for pt in (0, 1):
                    tcnt = seg_part_tiles(r, seg, pt)
                    if tcnt == 0:
                        vbs[pt] = None
                        continue
                    vb = v_p.tile([P, vbw * P], F16, tag="vb")
                    if kmode != "nogather":
                        nc.gpsimd.dma_gather(
                            out_ap=vb[:, 0:tcnt * P].rearrange(
                                "p (t e) -> p t e", e=P),
                            in_ap=bases[pt],
                            idxs_ap=idx_sb[r][:, icol:icol + tcnt * 8],
                            num_idxs=tcnt * P,
                            num_idxs_reg=tcnt * P,
                            elem_size=H,
                            single_packet=False,
                        )
                    else:
                        nc.vector.memset(vb[:, 0:tcnt * P], 0.0)
                    icol += tcnt * 8
                    vbs[pt] = vb
                ps = pp_hop.tile([P, WIN_PER_SEG * P], F32, space="PSUM",
                                 tag="hop")
                for wl, wi in enumerate(seg):
                    total = int(sum(Rr[wi, pt] + Tt[wi, pt] for pt in (0, 1)))
                    reg = ps[:, wl * P:(wl + 1) * P]
                    if total == 0:
                        nc.vector.memset(reg, 0.0)
                        continue
                    diag = m_p.tile([P, P], F16, tag="diag")
                    nc.vector.tensor_scalar(
                        out=diag[:], in0=ident[:],
                        scalar1=dinv_sb[r][:, wi:wi + 1], scalar2=None,
                        op0=mybir.AluOpType.mult)
                    kk = 0
                    for pt in (0, 1):
                        vb = vbs[pt]
                        loc0 = to[(wi, pt)] - to[(seg[0], pt)]
                        for t in range(int(Rr[wi, pt])):
                            nc.tensor.matmul(
                                out=reg,
                                lhsT=vb[:, (loc0 + t) * P:(loc0 + t + 1) * P],
                                rhs=diag[:],
                                start=(kk == 0), stop=(kk == total - 1))
                            kk += 1
                        base_t = loc0 + int(Rr[wi, pt])
                        for t in range(int(Tt[wi, pt])):
                            col = ta[(wi, pt)] + t
                            m = m_p.tile([P, P], F16, tag="onehot")
                            nc.vector.tensor_scalar(
                                out=m[:], in0=iota_f[:],
                                scalar1=dq_sb[r][:, col:col + 1],
                                scalar2=wq_sb[r][:, col:col + 1],
                                op0=mybir.AluOpType.is_equal,
                                op1=mybir.AluOpType.mult)
                            nc.tensor.matmul(
                                out=reg,
                                lhsT=vb[:, (base_t + t) * P:(base_t + t + 1) * P],
                                rhs=m[:],
                                start=(kk == 0), stop=(kk == total - 1))
                            kk += 1
                # epilogue: T_next(seg) = prev(seg) - ps; fused W3; nm build
                if t1_out is not None:
                    tgt = t1_out[:, j0 * P:j0 * P + segw]
                    nc.vector.tensor_tensor(out=tgt,
                                            in0=prev_fm[:, j0 * P:j0 * P + segw],
                                            in1=ps[:, 0:segw],
                                            op=mybir.AluOpType.subtract)
                    w3_accum_seg(tgt, k_w3, j0 * P, segw)
                    for wi in seg:
                        transpose_scale_window(t1_out, r, wi, nm_tile)
                else:
                    t2s = t2_p.tile([P, WIN_PER_SEG * P], F16, tag="t2s")
                    nc.vector.tensor_tensor(out=t2s[:, 0:segw],
                                            in0=prev_fm[:, j0 * P:j0 * P + segw],
                                            in1=ps[:, 0:segw],
                                            op=mybir.AluOpType.subtract)
                    w3_accum_seg(t2s[:, 0:segw], k_w3, j0 * P, segw)
            return nm_tile

        # ---- phase 1: MLPs for all relations (xT streamed per chunk) ----
        for (c0, cw) in mlp_chunks:
            xa = x_p.tile([P, MLP_CHUNK], F16, tag="xa")
            xb = x_p.tile([P, MLP_CHUNK], F16, tag="xb")
            xtmp = x_p.tile([P, MLP_CHUNK], F32, tag="xtmp")
            nc.sync.dma_start(out=xtmp[:, 0:cw], in_=xT_d[0:P, c0:c0 + cw])
            nc.any.tensor_copy(xa[:, 0:cw], xtmp[:, 0:cw])
            xtmp2 = x_p.tile([P, MLP_CHUNK], F32, tag="xtmp")
            nc.sync.dma_start(out=xtmp2[:, 0:cw], in_=xT_d[P:2 * P, c0:c0 + cw])
            nc.any.tensor_copy(xb[:, 0:cw], xtmp2[:, 0:cw])
            for r in range(R):
                ps1 = pp_big.tile([P, MLP_CHUNK], F32, space="PSUM", tag="big")
                nc.tensor.matmul(out=ps1[:, 0:cw], lhsT=W1a[r][:],
                                 rhs=xa[:, 0:cw], start=True, stop=False)
                nc.tensor.matmul(out=ps1[:, 0:cw], lhsT=W1b[r][:],
                                 rhs=xb[:, 0:cw], start=False, stop=True)
                h1 = h1_p.tile([P, MLP_CHUNK], F16, tag="h1")
                nc.scalar.activation(h1[:, 0:cw], ps1[:, 0:cw],
                                     mybir.ActivationFunctionType.Lrelu,
                                     bias=b1c[r][:], scale=1.0, alpha=0.01)
                ps2 = pp_big.tile([P, MLP_CHUNK], F32, space="PSUM", tag="big")
                nc.tensor.matmul(out=ps2[:, 0:cw], lhsT=W2sb[r][:],
                                 rhs=h1[:, 0:cw], start=True, stop=True)
                nc.scalar.activation(hT[r][:, c0:c0 + cw], ps2[:, 0:cw],
                                     mybir.ActivationFunctionType.Lrelu,
                                     bias=b2c[r][:], scale=1.0, alpha=0.01)

        # ---- phase 2: scaled transpose + AllGather h tables; W3p0 ----
        for r in range(R):
            nm = nm_p.tile([P, NLOC], F16, tag="nm")
            for wi in range(NWIN):
                transpose_scale_window(hT[r], r, wi, nm)
            store_and_allgather(nm, aghin[r], htab[r])
            for (c0, cw) in mlp_chunks:
                w3_accum_seg(hT[r][:, c0:c0 + cw], 0, c0, cw, first=(r == 0))

        # ---- phase 3: hop1 per relation; AG T1 tables ----
        kmode = os.environ.get("KMODE", "full")
        for r in range(R):
            if kmode == "nohop":
                nc.any.tensor_copy(T1[r][:], hT[r][:])
                for (c0, cw) in mlp_chunks:
                    w3_accum_seg(T1[r][:, c0:c0 + cw], 1, c0, cw)
                continue
            nm2 = hop(r, htab[r], hT[r], 1, T1[r])
            store_and_allgather(nm2, agtin[r], ttab[r])

        # ---- phase 4: hop2 per relation (fused, no T2 materialization) ----
        for r in range(R):
            if kmode == "nohop":
                for (c0, cw) in mlp_chunks:
                    w3_accum_seg(T1[r][:, c0:c0 + cw], 2, c0, cw)
                continue
            hop(r, ttab[r], T1[r], 2, None)

        # ---- output: leaky(out_acc + 3*b3), feat-major ----
        for (c0, cw) in mlp_chunks:
            oc = oc_p.tile([P, MLP_CHUNK], F32, tag="oc")
            nc.scalar.activation(oc[:, 0:cw], out_acc[:, c0:c0 + cw],
                                 mybir.ActivationFunctionType.Lrelu,
                                 bias=b3x3[:], scale=1.0, alpha=0.01)
            nc.sync.dma_start(out=out_d[:, c0:c0 + cw], in_=oc[:, 0:cw])

    nc.compile()
    return nc


# ----------------------------------------------------------------------------
# Entry point
# ----------------------------------------------------------------------------

_prog_cache = {}


def kernel(**inputs):
    meta, in_maps, perm = preprocess(inputs)
    key = repr((meta["N"], meta["NLOC"], meta["R"], meta["T"],
                meta["ntiles"], meta["ntail"]))
    if key not in _prog_cache:
        _prog_cache[key] = build_program(meta)
    nc = _prog_cache[key]
    res = run_bass_kernel_spmd(nc, in_maps, list(range(NCORES)))
    outs = [res.results[c]["out"] for c in range(NCORES)]  # [P, NLOC] each
    out_slots = np.concatenate(outs, axis=1).T             # [NPAD, H]
    n = meta["N"]
    return np.ascontiguousarray(out_slots[perm[:n]]).astype(np.float32)


# revision 11
# speedup vs baseline: 1.1016x; 1.1016x over previous
"""Trainium2 Bass kernel for nn_CombinedModel (3-relation GNN with Bernstein
polynomial message passing).

v2: identity-round scatter. dinv[src] is folded into the AllGathered table
(scaled during the PSUM->nm transpose copy) and dinv[dst] into a per-window
diagonal rhs, so most edges are aggregated by matmuls against a constant
diagonal (no per-tile one-hot build). Host assigns each edge of dst slot q to
round r (its r-th edge), so round tiles are slot-aligned: gathered row e
belongs to dst slot e. Empty slots gather a dedicated zero row (ghost slot,
zeroed via dinv=0). Leftover edges (beyond the per-window round count R) use
the one-hot path (is_equal vs iota, scaled by dinv[dst]), pinned to DVE.
W3 projections are fused into the per-seg hop epilogue; T2 is never
materialized in full.
"""
import math
import os
from contextlib import ExitStack

import numpy as np

import concourse.bacc as bacc
import concourse.tile as tile
from concourse import mybir
from concourse.bass_utils import run_bass_kernel_spmd
from concourse.masks import make_identity

F16, F32 = mybir.dt.float16, mybir.dt.float32
I16, I32 = mybir.dt.int16, mybir.dt.int32

NCORES = 8
P = 128
H = 128
IN_FEATS = 256
R = 3
D_ORDER = 2
KORD = D_ORDER + 1
WIN_PER_SEG = 4
MLP_CHUNK = 512
SPLIT = 25600        # flat-row lo/hi split (int16 index range)
Q0 = 0.5             # min fill fraction for an identity round


def _bernstein_thetas(d):
    thetas = []
    for i in range(d + 1):
        a = np.zeros(i + 1)
        a[i] = 0.5 ** i
        b = np.array([math.comb(d - i, j) * (-0.5) ** j for j in range(d - i + 1)])
        scale = math.factorial(d + 1) / (math.factorial(i) * math.factorial(d - i))
        thetas.append((np.convolve(a, b) * scale).astype(np.float32))
    return np.stack(thetas)  # [d+1, d+1]


THETAS = _bernstein_thetas(D_ORDER)


# ----------------------------------------------------------------------------
# Host-side preprocessing
# ----------------------------------------------------------------------------

def _make_plan(n):
    nloc = ((n + NCORES * P - 1) // (NCORES * P)) * P
    npad = nloc * NCORES
    nwin = nloc // P
    segs = [list(range(s, min(s + WIN_PER_SEG, nwin)))
            for s in range(0, nwin, WIN_PER_SEG)]
    return dict(N=n, NLOC=nloc, NPAD=npad, NWIN=nwin, segs=segs)


def _build_perm(degs_total, npad):
    """Balanced node -> slot permutation (snake-deal by total degree), with
    the last slot of the last window of each core reserved for a ghost
    (zero) node used as the gather target for empty round slots."""
    n = len(degs_total)
    nloc = npad // NCORES
    nwin = nloc // P
    tot = np.zeros(npad, np.int64)
    tot[:n] = degs_total
    order = np.argsort(-tot, kind="stable")
    slot_of = np.empty(npad, np.int64)
    counts = np.zeros(npad // P, np.int64)
    i = np.arange(npad)
    rnd, pos = np.divmod(i, npad // P)
    w = np.where(rnd % 2 == 0, pos, npad // P - 1 - pos)
    core = w % NCORES
    j = w // NCORES
    for idx in range(npad):
        g = order[idx]
        ww = w[idx]
        slot_of[g] = core[idx] * nloc + j[idx] * P + counts[ww]
        counts[ww] += 1
    # reserve ghost slots: swap ghosts (ids npad-1..npad-NCORES) into the
    # fixed slot (core c, window nwin-1, q=P-1)
    assert npad - n >= NCORES
    inv = np.empty(npad, np.int64)
    inv[slot_of] = np.arange(npad)
    for c in range(NCORES):
        tgt = c * nloc + (nwin - 1) * P + (P - 1)
        gid = npad - 1 - c
        cur = slot_of[gid]
        if cur == tgt:
            continue
        other = inv[tgt]
        slot_of[other], slot_of[gid] = cur, tgt
        inv[cur], inv[tgt] = other, gid
    return slot_of


def _pack_idx(idx_flat):
    """[L] int16 -> wrapped [128, L//16] layout (16-partition wrap, replicated)."""
    L = len(idx_flat)
    assert L % 16 == 0
    base = idx_flat.reshape(L // 16, 16).T  # [16, L/16]
    return np.ascontiguousarray(np.tile(base, (8, 1))).astype(np.int16)


def _flat_row(slot, nloc, nwin):
    """node slot (global) -> flat 256B-row index in the wrapped AG table.

    slot = c*nloc + j*128 + q  ->  row (c*128 + q)*nwin + j
    """
    c = slot // nloc
    l = slot % nloc
    j = l // P
    q = l % P
    return (c * P + q) * nwin + j


def _ghost_rows(nwin):
    lo = (0 * P + (P - 1)) * nwin + (nwin - 1)
    hi = ((NCORES - 1) * P + (P - 1)) * nwin + (nwin - 1) - SPLIT
    assert 0 <= lo < SPLIT and 0 <= hi < 32768
    return lo, hi


def _build_streams(plan, s_slot, d_slot, dinv_dst_edge):
    """Identity-round + one-hot-tail streams for one relation.

    Returns (Rr [nwin,2], Tt [nseg,2], ntiles, ntail, per_core list of
    dict(idx, dq, wq)). Stream tile order: for seg: for part:
    [for w in seg: R(w,part) ident tiles][T(seg,part) tail tiles].
    Ident tile r holds, at partition q, the src row of dst-slot q's r-th
    edge on that part (or the ghost zero row). Tail edges are compacted at
    seg granularity (dq = dst offset within seg, wq = dinv[dst]).
    """
    NLOC, NWIN, segs = plan["NLOC"], plan["NWIN"], plan["segs"]
    nseg = len(segs)
    ghost_lo, ghost_hi = _ghost_rows(NWIN)
    ghost = (ghost_lo, ghost_hi)

    srow = _flat_row(s_slot, NLOC, NWIN)
    part = (srow >= SPLIT).astype(np.int64)
    ival = (srow - part * SPLIT).astype(np.int64)
    core = d_slot // NLOC
    lloc = d_slot % NLOC
    w = lloc // P
    q = lloc % P
    seg_of = w // WIN_PER_SEG
    seg0 = np.array([s[0] for s in segs], np.int64)
    soff = (lloc - seg0[seg_of] * P).astype(np.float32)  # dst offset in seg

    key = ((core * NWIN + w) * 2 + part) * P + q
    nkey = NCORES * NWIN * 2 * P
    cnt = np.bincount(key, minlength=nkey).reshape(NCORES, NWIN, 2, P)

    Rr = np.zeros((NWIN, 2), np.int64)
    for wi in range(NWIN):
        for pt in (0, 1):
            cs = cnt[:, wi, pt, :]
            r = 0
            while (cs >= r + 1).sum() >= Q0 * NCORES * P:
                r += 1
            Rr[wi, pt] = r
    # leftover pooled at seg level
    lo_w = np.maximum(cnt - Rr[None, :, :, None], 0).sum(axis=3)  # [C,NWIN,2]
    lo_seg = np.zeros((NCORES, nseg, 2), np.int64)
    for si, seg in enumerate(segs):
        lo_seg[:, si, :] = lo_w[:, seg, :].sum(axis=1)
    Tt = np.ceil(lo_seg / P).astype(np.int64).max(axis=0)  # [nseg,2]

    tile_off = {}   # (wi, pt) -> ident tile base
    tail_tile = {}  # (si, pt) -> tail tile base
    tail_col = {}   # (si, pt) -> tail dq/wq column base
    tpos = tc = 0
    for si, seg in enumerate(segs):
        for pt in (0, 1):
            for wi in seg:
                tile_off[(wi, pt)] = tpos
                tpos += int(Rr[wi, pt])
            tail_tile[(si, pt)] = tpos
            tail_col[(si, pt)] = tc
            tpos += int(Tt[si, pt])
            tc += int(Tt[si, pt])
    ntiles, ntail = tpos, tc

    # rank of each edge within its (c,w,part,q) group, ordered by src row
    order = np.lexsort((ival, q, part, w, core))
    k_sorted = key[order]
    new_grp = np.r_[True, np.diff(k_sorted) != 0]
    starts = np.flatnonzero(new_grp)
    grp_id = np.cumsum(new_grp) - 1
    rank = np.arange(len(order)) - starts[grp_id]

    per_core = []
    for c in range(NCORES):
        mask = core[order] == c
        sel = order[mask]
        rk = rank[mask]
        idx_c = np.empty(ntiles * P, np.int64)
        for (wi, pt), t0 in tile_off.items():
            idx_c[t0 * P:(t0 + int(Rr[wi, pt])) * P] = ghost[pt]
        for (si, pt), t0 in tail_tile.items():
            idx_c[t0 * P:(t0 + int(Tt[si, pt])) * P] = ghost[pt]
        dq_c = np.full(max(ntail, 1) * P, -1.0, np.float32)
        wq_c = np.zeros(max(ntail, 1) * P, np.float32)

        is_ident = rk < Rr[w[sel], part[sel]]
        e_id = sel[is_ident]
        if len(e_id):
            pos = (np.array([tile_off[(wi, pt)] for wi, pt in
                             zip(w[e_id], part[e_id])], np.int64)
                   + rk[is_ident]) * P + q[e_id]
            idx_c[pos] = ival[e_id]

        e_tail = sel[~is_ident]
        if len(e_tail):
            # group tails by (seg, part); order within group by src row
            tkey = seg_of[e_tail] * 2 + part[e_tail]
            t_ord = np.lexsort((ival[e_tail], tkey))
            e_tail = e_tail[t_ord]
            tkey = tkey[t_ord]
            tnew = np.r_[True, np.diff(tkey) != 0]
            tstarts = np.flatnonzero(tnew)
            tgrp = np.cumsum(tnew) - 1
            tr = np.arange(len(e_tail)) - tstarts[tgrp]
            base_tile = np.array(
                [tail_tile[(si, pt)] for si, pt in
                 zip(seg_of[e_tail], part[e_tail])], np.int64)
            tpos_e = (base_tile + tr // P) * P + tr % P
            idx_c[tpos_e] = ival[e_tail]
            base_col = np.array(
                [tail_col[(si, pt)] for si, pt in
                 zip(seg_of[e_tail], part[e_tail])], np.int64)
            cpos = (base_col + tr // P) * P + tr % P
            dq_c[cpos] = soff[e_tail]
            wq_c[cpos] = dinv_dst_edge[e_tail]

        per_core.append(dict(
            idx=_pack_idx(idx_c.astype(np.int16)),
            dq=np.ascontiguousarray(dq_c.reshape(max(ntail, 1), P).T),
            wq=np.ascontiguousarray(wq_c.reshape(max(ntail, 1), P).T),
        ))
    return Rr, Tt, ntiles, ntail, per_core


def preprocess(inputs):
    x = np.asarray(inputs["x"], np.float32)
    n = x.shape[0]
    plan = _make_plan(n)
    NLOC, NPAD, NWIN = plan["NLOC"], plan["NPAD"], plan["NWIN"]

    srcs, dsts, degs = [], [], []
    for r in range(R):
        s = np.asarray(inputs[f"src{r}"]).astype(np.int64)
        d = np.asarray(inputs[f"dst{r}"]).astype(np.int64)
        srcs.append(s)
        dsts.append(d)
        degs.append(np.bincount(d, minlength=n).astype(np.float64))
    perm = _build_perm(sum(degs)[:n].astype(np.int64), NPAD)  # global -> slot
    inv_perm = np.empty(NPAD, np.int64)
    inv_perm[perm] = np.arange(NPAD)

    meta = dict(N=n, NLOC=NLOC, NPAD=NPAD, NWIN=NWIN,
                segs=tuple(tuple(s) for s in plan["segs"]))
    Rs_l, Ts_l, ntiles_l, ntail_l, streams, dinv_nms = [], [], [], [], [], []
    for r in range(R):
        dinv = (1.0 / np.sqrt(np.maximum(degs[r], 1.0))).astype(np.float32)
        Rr, Tt, ntiles, ntail, per_core = _build_streams(
            plan, perm[srcs[r]], perm[dsts[r]], dinv[dsts[r]])
        Rs_l.append(tuple(tuple(int(v) for v in row) for row in Rr))
        Ts_l.append(tuple(tuple(int(v) for v in row) for row in Tt))
        ntiles_l.append(ntiles)
        ntail_l.append(max(ntail, 1))
        streams.append(per_core)
        # per-core dinv in wrapped (q, window) layout; 0 for pad slots
        dinv_pad = np.zeros(NPAD, np.float32)
        dinv_pad[:n] = dinv[:n]
        dn = []
        for c in range(NCORES):
            slots = (c * NLOC + np.arange(NLOC)).reshape(NWIN, P)
            vals = dinv_pad[np.minimum(inv_perm[slots], n - 1)]
            vals[inv_perm[slots] >= n] = 0.0
            dn.append(np.ascontiguousarray(vals.T.astype(np.float32)))  # [P,NWIN]
        dinv_nms.append(dn)
    meta["R"] = tuple(Rs_l)
    meta["T"] = tuple(Ts_l)
    meta["ntiles"] = tuple(ntiles_l)
    meta["ntail"] = tuple(ntail_l)

    x_slots = np.zeros((NPAD, IN_FEATS), np.float32)
    x_slots[perm[:n]] = x
    in_maps = []
    weight_names = []
    for r in range(R):
        weight_names += [f"W1_{r}", f"b1_{r}", f"W2_{r}", f"b2_{r}"]
    weight_names += ["W3", "b3"]
    for c in range(NCORES):
        m = {"xT": np.ascontiguousarray(
            x_slots[c * NLOC:(c + 1) * NLOC].T)}
        for name in weight_names:
            m[name] = np.asarray(inputs[name], np.float32)
        for r in range(R):
            m[f"idx{r}"] = streams[r][c]["idx"]
            m[f"dq{r}"] = streams[r][c]["dq"]
            m[f"wq{r}"] = streams[r][c]["wq"]
            m[f"dinv{r}"] = dinv_nms[r][c]
        in_maps.append(m)
    return meta, in_maps, perm


# ----------------------------------------------------------------------------
# Device program
# ----------------------------------------------------------------------------

def build_program(meta):
    NLOC, NPAD, NWIN = meta["NLOC"], meta["NPAD"], meta["NWIN"]
    segs = [list(s) for s in meta["segs"]]
    Rs = [np.array(t, np.int64) for t in meta["R"]]
    Ts = [np.array(t, np.int64) for t in meta["T"]]
    ntiles = meta["ntiles"]
    ntail = meta["ntail"]

    # per-(seg,part) gather tile counts and max for vb sizing
    def seg_part_tiles(r, si, pt):
        return int(sum(Rs[r][wi, pt] for wi in segs[si]) + Ts[r][si, pt])

    vbw = 1
    for r in range(R):
        for si in range(len(segs)):
            for pt in (0, 1):
                vbw = max(vbw, seg_part_tiles(r, si, pt))

    # tile/tail offset bookkeeping (must match host stream order)
    tile_offs, tail_tiles, tail_cols = [], [], []
    for r in range(R):
        to, tt, tcl = {}, {}, {}
        tpos = tc = 0
        for si, seg in enumerate(segs):
            for pt in (0, 1):
                for wi in seg:
                    to[(wi, pt)] = tpos
                    tpos += int(Rs[r][wi, pt])
                tt[(si, pt)] = tpos
                tcl[(si, pt)] = tc
                tpos += int(Ts[r][si, pt])
                tc += int(Ts[r][si, pt])
        assert tpos == ntiles[r]
        tile_offs.append(to)
        tail_tiles.append(tt)
        tail_cols.append(tcl)

    nc = bacc.Bacc("TRN2", target_bir_lowering=False, debug=False,
                   num_devices=NCORES)

    xT_d = nc.dram_tensor("xT", [IN_FEATS, NLOC], F32, kind="ExternalInput").ap()
    Wd = {}
    for r in range(R):
        Wd[f"W1_{r}"] = nc.dram_tensor(f"W1_{r}", [IN_FEATS, H], F32, kind="ExternalInput").ap()
        Wd[f"b1_{r}"] = nc.dram_tensor(f"b1_{r}", [H], F32, kind="ExternalInput").ap()
        Wd[f"W2_{r}"] = nc.dram_tensor(f"W2_{r}", [H, H], F32, kind="ExternalInput").ap()
        Wd[f"b2_{r}"] = nc.dram_tensor(f"b2_{r}", [H], F32, kind="ExternalInput").ap()
    W3_d = nc.dram_tensor("W3", [KORD * H, H], F32, kind="ExternalInput").ap()
    b3_d = nc.dram_tensor("b3", [H], F32, kind="ExternalInput").ap()
    idx_d, dq_d, wq_d, dinv_d = [], [], [], []
    for r in range(R):
        idx_d.append(nc.dram_tensor(f"idx{r}", [P, ntiles[r] * 8], I16, kind="ExternalInput").ap())
        dq_d.append(nc.dram_tensor(f"dq{r}", [P, ntail[r]], F32, kind="ExternalInput").ap())
        wq_d.append(nc.dram_tensor(f"wq{r}", [P, ntail[r]], F32, kind="ExternalInput").ap())
        dinv_d.append(nc.dram_tensor(f"dinv{r}", [P, NWIN], F32, kind="ExternalInput").ap())
    out_d = nc.dram_tensor("out", [P, NLOC], F32, kind="ExternalOutput").ap()

    aghin, htab, agtin, ttab = [], [], [], []
    for r in range(R):
        aghin.append(nc.dram_tensor(f"aghin{r}", [P, NLOC], F16))
        htab.append(nc.dram_tensor(f"htab{r}", [NCORES * P, NLOC], F16,
                                   addr_space="Shared"))
        agtin.append(nc.dram_tensor(f"agtin{r}", [P, NLOC], F16))
        ttab.append(nc.dram_tensor(f"ttab{r}", [NCORES * P, NLOC], F16,
                                   addr_space="Shared"))

    mlp_chunks = []
    c0 = 0
    while c0 < NLOC:
        cw = min(MLP_CHUNK, NLOC - c0)
        mlp_chunks.append((c0, cw))
        c0 += cw

    with tile.TileContext(nc) as tc_, ExitStack() as ctx:
        consts = ctx.enter_context(tc_.tile_pool(name="consts", bufs=1))
        wtmp_p = ctx.enter_context(tc_.tile_pool(name="wtmp", bufs=2))
        nm_p = ctx.enter_context(tc_.tile_pool(name="nm", bufs=1))
        idx_p = ctx.enter_context(tc_.tile_pool(name="idxp", bufs=1))
        dq_p = ctx.enter_context(tc_.tile_pool(name="dqp", bufs=1))
        v_p = ctx.enter_context(tc_.tile_pool(name="vp", bufs=3))
        m_p = ctx.enter_context(tc_.tile_pool(name="mp", bufs=3))
        dg_p = ctx.enter_context(tc_.tile_pool(name="dgp", bufs=3))
        x_p = ctx.enter_context(tc_.tile_pool(name="xp", bufs=2))
        h1_p = ctx.enter_context(tc_.tile_pool(name="h1p", bufs=2))
        t2_p = ctx.enter_context(tc_.tile_pool(name="t2p", bufs=2))
        oc_p = ctx.enter_context(tc_.tile_pool(name="ocp", bufs=1))
        cast_p = ctx.enter_context(tc_.tile_pool(name="castp", bufs=1))
        pp_big = ctx.enter_context(tc_.tile_pool(name="ppbig", bufs=3, space="PSUM"))
        pp_hop = ctx.enter_context(tc_.tile_pool(name="pphop", bufs=3, space="PSUM"))
        pp_tr = ctx.enter_context(tc_.tile_pool(name="pptr", bufs=2, space="PSUM"))

        # ---- constants ----
        iota_i = wtmp_p.tile([P, WIN_PER_SEG * P], I32, tag="iotai")
        nc.gpsimd.iota(iota_i[:], pattern=[[1, WIN_PER_SEG * P]], base=0,
                       channel_multiplier=0)
        iota_f = consts.tile([P, WIN_PER_SEG * P], F16, tag="iotaf")
        nc.vector.tensor_copy(iota_f[:], iota_i[:])
        ident = consts.tile([P, P], F16, tag="ident")
        make_identity(nc, ident[:])

        def load_cast(dst, src_ap, ncols):
            c0 = 0
            while c0 < ncols:
                cw = min(512, ncols - c0)
                tmp = cast_p.tile([P, 512], F32, tag="cast")
                nc.sync.dma_start(out=tmp[:, 0:cw], in_=src_ap[:, c0:c0 + cw])
                nc.any.tensor_copy(dst[:, c0:c0 + cw], tmp[:, 0:cw])
                c0 += cw

        W1a, W1b, W2sb, b1c, b2c = [], [], [], [], []
        for r in range(R):
            wa = consts.tile([P, H], F16, tag=f"w1a{r}")
            wb = consts.tile([P, H], F16, tag=f"w1b{r}")
            w2 = consts.tile([P, H], F16, tag=f"w2{r}")
            load_cast(wa, Wd[f"W1_{r}"][0:P, :], H)
            load_cast(wb, Wd[f"W1_{r}"][P:2 * P, :], H)
            load_cast(w2, Wd[f"W2_{r}"][:, :], H)
            b1 = consts.tile([P, 1], F32, tag=f"b1{r}")
            b2 = consts.tile([P, 1], F32, tag=f"b2{r}")
            nc.sync.dma_start(out=b1[:], in_=Wd[f"b1_{r}"][:, None])
            nc.sync.dma_start(out=b2[:], in_=Wd[f"b2_{r}"][:, None])
            W1a.append(wa); W1b.append(wb); W2sb.append(w2)
            b1c.append(b1); b2c.append(b2)

        # W3 folded by Bernstein thetas: W3p_k = sum_j THETA[j,k] * W3_j
        w3s = []
        for jj in range(KORD):
            t = wtmp_p.tile([P, H], F32, tag=f"w3s{jj}")
            nc.sync.dma_start(out=t[:], in_=W3_d[jj * H:(jj + 1) * H, :])
            w3s.append(t)
        W3p = []
        for k in range(KORD):
            acc = wtmp_p.tile([P, H], F32, tag=f"w3acc{k}")
            nc.vector.tensor_scalar(out=acc[:], in0=w3s[0][:],
                                    scalar1=float(THETAS[0, k]), scalar2=None,
                                    op0=mybir.AluOpType.mult)
            for jj in range(1, KORD):
                t2t = wtmp_p.tile([P, H], F32, tag="w3mul")
                nc.vector.tensor_scalar(out=t2t[:], in0=w3s[jj][:],
                                        scalar1=float(THETAS[jj, k]), scalar2=None,
                                        op0=mybir.AluOpType.mult)
                nc.vector.tensor_tensor(out=acc[:], in0=acc[:], in1=t2t[:],
                                        op=mybir.AluOpType.add)
            wk = consts.tile([P, H], F16, tag=f"w3p{k}")
            nc.vector.tensor_copy(wk[:], acc[:])
            W3p.append(wk)
        b3x3 = consts.tile([P, 1], F32, tag="b3x3")
        nc.sync.dma_start(out=b3x3[:], in_=b3_d[:, None])
        nc.vector.tensor_scalar(out=b3x3[:], in0=b3x3[:], scalar1=3.0,
                                scalar2=None, op0=mybir.AluOpType.mult)

        out_acc = consts.tile([P, NLOC], F16, tag="outacc")

        hT, T1, dinv_sb = [], [], []
        for r in range(R):
            ht_r = consts.tile([P, NLOC], F16, tag=f"ht{r}")
            t1_r = consts.tile([P, NLOC], F16, tag=f"t1{r}")
            dv = consts.tile([P, NWIN], F32, tag=f"dinv{r}")
            nc.sync.dma_start(out=dv[:], in_=dinv_d[r][:])
            hT.append(ht_r)
            T1.append(t1_r)
            dinv_sb.append(dv)

        idx_sb, dq_sb, wq_sb = [], [], []
        for r in range(R):
            idx_r = idx_p.tile([P, ntiles[r] * 8], I16, tag=f"idx{r}")
            dq_r = dq_p.tile([P, ntail[r]], F32, tag=f"dq{r}")
            wq_r = dq_p.tile([P, ntail[r]], F32, tag=f"wq{r}")
            idx_sb.append(idx_r)
            dq_sb.append(dq_r)
            wq_sb.append(wq_r)
        for r in range(R):
            nc.sync.dma_start(out=idx_sb[r][:], in_=idx_d[r][:])
            nc.sync.dma_start(out=dq_sb[r][:], in_=dq_d[r][:])
            nc.sync.dma_start(out=wq_sb[r][:], in_=wq_d[r][:])

        def transpose_scale_window(src_fm, r, wi, nm_tile):
            """nm[:, wi*P:(wi+1)*P] = dinv[:,wi] * transpose(src window)."""
            tp = pp_tr.tile([P, P], F16, space="PSUM", tag="tr")
            nc.tensor.transpose(out=tp[:], in_=src_fm[:, wi * P:(wi + 1) * P],
                                identity=ident[:])
            nc.any.tensor_scalar(out=nm_tile[:, wi * P:(wi + 1) * P],
                                 in0=tp[:], scalar1=dinv_sb[r][:, wi:wi + 1],
                                 scalar2=None, op0=mybir.AluOpType.mult)

        def store_and_allgather(nm_tile, ag_in, table):
            nc.sync.dma_start(out=ag_in.ap(), in_=nm_tile[:])
            if os.environ.get("KNOCC"):
                return
            nc.gpsimd.collective_compute(
                "AllGather", mybir.AluOpType.bypass,
                ins=[ag_in.ap()], outs=[table.ap()],
                replica_groups=[list(range(NCORES))])

        def w3_accum_seg(src_ap, k, cols0, cw, first=False):
            psf = pp_big.tile([P, MLP_CHUNK], F32, space="PSUM", tag="big")
            nc.tensor.matmul(out=psf[:, 0:cw], lhsT=W3p[k][:], rhs=src_ap,
                             start=True, stop=True)
            if first:
                nc.any.tensor_copy(out_acc[:, cols0:cols0 + cw], psf[:, 0:cw])
            else:
                nc.any.tensor_tensor(out=out_acc[:, cols0:cols0 + cw],
                                     in0=out_acc[:, cols0:cols0 + cw],
                                     in1=psf[:, 0:cw],
                                     op=mybir.AluOpType.add)

        def hop(r, table, prev_fm, k_w3, t1_out):
            """One hop: gather + diag/one-hot matmuls + fused epilogue.

            If t1_out is not None: writes T1 and also builds+stores the
            scaled nm for the next AllGather (returns nm tile). Otherwise
            uses a transient seg tile (hop2) and only accumulates W3.
            """
            Rr, Tt = Rs[r], Ts[r]
            to, tt, tcl = tile_offs[r], tail_tiles[r], tail_cols[r]
            kmode = os.environ.get("KMODE", "full")
            nm_tile = None
            if t1_out is not None:
                nm_tile = nm_p.tile([P, NLOC], F16, tag="nm")
            flat = table.ap().rearrange("a (j f) -> (a j) f", f=H)
            bases = (flat[0:SPLIT, :], flat[SPLIT:NPAD, :])
            icol = 0
            for si, seg in enumerate(segs):
                segw = len(seg) * P
                j0 = seg[0]
                vbs = {}
                sp_base = {}
                for pt in (0, 1):
                    tcnt = seg_part_tiles(r, si, pt)
                    sp_base[pt] = to[(seg[0], pt)]
                    if tcnt == 0:
                        vbs[pt] = None
                        continue
                    vb = v_p.tile([P, vbw * P], F16, tag="vb")
                    if kmode != "nogather":
                        nc.gpsimd.dma_gather(
                            out_ap=vb[:, 0:tcnt * P].rearrange(
                                "p (t e) -> p t e", e=P),
                            in_ap=bases[pt],
                            idxs_ap=idx_sb[r][:, icol:icol + tcnt * 8],
                            num_idxs=tcnt * P,
                            num_idxs_reg=tcnt * P,
                            elem_size=H,
                            single_packet=False,
                        )
                    else:
                        nc.vector.memset(vb[:, 0:tcnt * P], 0.0)
                    icol += tcnt * 8
                    vbs[pt] = vb
                ps = pp_hop.tile([P, WIN_PER_SEG * P], F32, space="PSUM",
                                 tag="hop")
                total = int(sum(Rr[wi, pt] for wi in seg for pt in (0, 1))
                            + Tt[si, 0] + Tt[si, 1])
                if total == 0:
                    nc.vector.memset(ps[:, 0:segw], 0.0)
                kk = 0
                # tails first: the first tail writes the full seg width,
                # consuming the whole bank's pending-zero in one go
                for pt in (0, 1):
                    for t in range(int(Tt[si, pt])):
                        col = tcl[(si, pt)] + t
                        m = m_p.tile([P, WIN_PER_SEG * P], F16, tag="onehot")
                        nc.vector.tensor_scalar(
                            out=m[:, 0:segw], in0=iota_f[:, 0:segw],
                            scalar1=dq_sb[r][:, col:col + 1],
                            scalar2=wq_sb[r][:, col:col + 1],
                            op0=mybir.AluOpType.is_equal,
                            op1=mybir.AluOpType.mult)
                        tb = tt[(si, pt)] - sp_base[pt]
                        nc.tensor.matmul(
                            out=ps[:, 0:segw],
                            lhsT=vbs[pt][:, (tb + t) * P:(tb + t + 1) * P],
                            rhs=m[:, 0:segw],
                            start=(kk == 0), stop=(kk == total - 1))
                        kk += 1
                no_tails = kk == 0
                for wl, wi in enumerate(seg):
                    widents = int(Rr[wi, 0] + Rr[wi, 1])
                    if widents == 0:
                        if no_tails:
                            nc.vector.memset(ps[:, wl * P:(wl + 1) * P], 0.0)
                        continue
                    diag = dg_p.tile([P, P], F16, tag="diag")
                    nc.vector.tensor_scalar(
                        out=diag[:], in0=ident[:],
                        scalar1=dinv_sb[r][:, wi:wi + 1], scalar2=None,
                        op0=mybir.AluOpType.mult)
                    for pt in (0, 1):
                        vb = vbs[pt]
                        loc0 = to[(wi, pt)] - sp_base[pt]
                        for t in range(int(Rr[wi, pt])):
                            nc.tensor.matmul(
                                out=ps[:, wl * P:(wl + 1) * P],
                                lhsT=vb[:, (loc0 + t) * P:(loc0 + t + 1) * P],
                                rhs=diag[:],
                                start=(kk == 0), stop=(kk == total - 1))
                            kk += 1
                # epilogue: T_next(seg) = prev(seg) - ps; fused W3; nm build
                if t1_out is not None:
                    tgt = t1_out[:, j0 * P:j0 * P + segw]
                    nc.vector.tensor_tensor(out=tgt,
                                            in0=prev_fm[:, j0 * P:j0 * P + segw],
                                            in1=ps[:, 0:segw],
                                            op=mybir.AluOpType.subtract)
                    w3_accum_seg(tgt, k_w3, j0 * P, segw)
                    for wi in seg:
                        transpose_scale_window(t1_out, r, wi, nm_tile)
                else:
                    t2s = t2_p.tile([P, WIN_PER_SEG * P], F16, tag="t2s")
                    nc.vector.tensor_tensor(out=t2s[:, 0:segw],
                                            in0=prev_fm[:, j0 * P:j0 * P + segw],
                                            in1=ps[:, 0:segw],
                                            op=mybir.AluOpType.subtract)
                    w3_accum_seg(t2s[:, 0:segw], k_w3, j0 * P, segw)
            return nm_tile

        # ---- phase 1: MLPs for all relations (xT streamed per chunk) ----
        for (c0, cw) in mlp_chunks:
            xa = x_p.tile([P, MLP_CHUNK], F16, tag="xa")
            xb = x_p.tile([P, MLP_CHUNK], F16, tag="xb")
            xtmp = x_p.tile([P, MLP_CHUNK], F32, tag="xtmp")
            nc.sync.dma_start(out=xtmp[:, 0:cw], in_=xT_d[0:P, c0:c0 + cw])
            nc.any.tensor_copy(xa[:, 0:cw], xtmp[:, 0:cw])
            xtmp2 = x_p.tile([P, MLP_CHUNK], F32, tag="xtmp")
            nc.sync.dma_start(out=xtmp2[:, 0:cw], in_=xT_d[P:2 * P, c0:c0 + cw])
            nc.any.tensor_copy(xb[:, 0:cw], xtmp2[:, 0:cw])
            for r in range(R):
                ps1 = pp_big.tile([P, MLP_CHUNK], F32, space="PSUM", tag="big")
                nc.tensor.matmul(out=ps1[:, 0:cw], lhsT=W1a[r][:],
                                 rhs=xa[:, 0:cw], start=True, stop=False)
                nc.tensor.matmul(out=ps1[:, 0:cw], lhsT=W1b[r][:],
                                 rhs=xb[:, 0:cw], start=False, stop=True)
                h1 = h1_p.tile([P, MLP_CHUNK], F16, tag="h1")
                nc.scalar.activation(h1[:, 0:cw], ps1[:, 0:cw],
                                     mybir.ActivationFunctionType.Lrelu,
                                     bias=b1c[r][:], scale=1.0, alpha=0.01)
                ps2 = pp_big.tile([P, MLP_CHUNK], F32, space="PSUM", tag="big")
                nc.tensor.matmul(out=ps2[:, 0:cw], lhsT=W2sb[r][:],
                                 rhs=h1[:, 0:cw], start=True, stop=True)
                nc.scalar.activation(hT[r][:, c0:c0 + cw], ps2[:, 0:cw],
                                     mybir.ActivationFunctionType.Lrelu,
                                     bias=b2c[r][:], scale=1.0, alpha=0.01)

        # ---- phase 2: scaled transpose + AllGather h tables; W3p0 ----
        for r in range(R):
            nm = nm_p.tile([P, NLOC], F16, tag="nm")
            for wi in range(NWIN):
                transpose_scale_window(hT[r], r, wi, nm)
            store_and_allgather(nm, aghin[r], htab[r])
            for (c0, cw) in mlp_chunks:
                w3_accum_seg(hT[r][:, c0:c0 + cw], 0, c0, cw, first=(r == 0))

        # ---- phase 3: hop1 per relation; AG T1 tables ----
        kmode = os.environ.get("KMODE", "full")
        for r in range(R):
            if kmode == "nohop":
                nc.any.tensor_copy(T1[r][:], hT[r][:])
                for (c0, cw) in mlp_chunks:
                    w3_accum_seg(T1[r][:, c0:c0 + cw], 1, c0, cw)
                continue
            nm2 = hop(r, htab[r], hT[r], 1, T1[r])
            store_and_allgather(nm2, agtin[r], ttab[r])

        # ---- phase 4: hop2 per relation (fused, no T2 materialization) ----
        for r in range(R):
            if kmode == "nohop":
                for (c0, cw) in mlp_chunks:
                    w3_accum_seg(T1[r][:, c0:c0 + cw], 2, c0, cw)
                continue
            hop(r, ttab[r], T1[r], 2, None)

        # ---- output: leaky(out_acc + 3*b3), feat-major ----
        for (c0, cw) in mlp_chunks:
            oc = oc_p.tile([P, MLP_CHUNK], F32, tag="oc")
            nc.scalar.activation(oc[:, 0:cw], out_acc[:, c0:c0 + cw],
                                 mybir.ActivationFunctionType.Lrelu,
                                 bias=b3x3[:], scale=1.0, alpha=0.01)
            nc.sync.dma_start(out=out_d[:, c0:c0 + cw], in_=oc[:, 0:cw])

    nc.compile()
    return nc


# ----------------------------------------------------------------------------
# Entry point
# ----------------------------------------------------------------------------

_prog_cache = {}


def _cache_key(meta):
    return repr((meta["N"], meta["NLOC"], meta["R"], meta["T"],
                 meta["ntiles"], meta["ntail"]))


def kernel(**inputs):
    meta, in_maps, perm = preprocess(inputs)
    key = _cache_key(meta)
    if key not in _prog_cache:
        _prog_cache[key] = build_program(meta)
    nc = _prog_cache[key]
    res = run_bass_kernel_spmd(nc, in_maps, list(range(NCORES)))
    outs = [res.results[c]["out"] for c in range(NCORES)]  # [P, NLOC] each
    out_slots = np.concatenate(outs, axis=1).T             # [NPAD, H]
    n = meta["N"]
    return np.ascontiguousarray(out_slots[perm[:n]]).astype(np.float32)
